# revision 1
# baseline (speedup 1.0000x reference)
"""Trainium2 Bass kernel for nn_Attention_8839042695176.

Full (unsharded) inputs in, full output out. Internally: 8 NeuronCores,
core h owns attention head h (both batch elements), convs/qkv replicated
per core on that core's permuted channel order.

Math per (b, h) unit:
    scores[i,j] = q_full[c,i]·emb[c,j] + qd_up[c,i]·kd_up[c,j]   (K=16 matmul)
    attn = softmax_j(scores)        (no max-subtraction; |scores| <~ 8)
    out[c,i]  = sum_j attn[i,j] vv[c,j]
computed in transposed layout E^T[j,i] so both big matmuls stream on PE,
with the softmax denominator fused in as an extra all-ones row of vv^T.
"""

import numpy as np

HEADS = 8
DIM_HEAD = 8
B = 2
C = 64
H = 48
HW = H * H          # 2304
KS = 11             # conv kernel
STRIDE = 8
PAD = 2
M6 = 6              # downsampled side
MM = M6 * M6        # 36
PADW = H + 2 * PAD  # 52
PADHW = PADW * PADW # 2704
SCALE = DIM_HEAD ** (-0.5)
NCORES = 8
TAPS = KS * KS      # 121

# i-chunks for the main loop (<=512 fp32 moving limit)
ICHUNKS = [(0, 512), (512, 512), (1024, 512), (1536, 512), (2048, 256)]
# chunks for the qkv projection, row-aligned to 48 (10 rows / 8 rows)
QCHUNKS = [(0, 480), (480, 480), (960, 480), (1440, 480), (1920, 384)]
NJT = HW // 128     # 18 j-tiles

_PROGRAMS = {}
# Conv activation selector: "Gelu" on hardware; CoreSim lacks Gelu, so the
# sim test swaps in "Tanh" (np reference adapted identically).
_CONV_ACT = "Gelu"
# When True, adds debug DRAM outputs for intermediates (sim debugging only).
_DEBUG = False


def _build_program(repeat=1, split=True):
    from contextlib import ExitStack
    import concourse.bass as bass
    import concourse.mybir as mybir
    import concourse.tile as tile
    from concourse.masks import make_identity

    F32 = mybir.dt.float32
    BF = mybir.dt.bfloat16
    AF = mybir.ActivationFunctionType

    nc = bass.Bass(trn_type="TRN2")

    f2 = nc.declare_dram_parameter("f2", [B, C, HW], BF, isOutput=False)
    w1T = nc.declare_dram_parameter("w1T", [C, 128], BF, isOutput=False)
    wvT = nc.declare_dram_parameter("wvT", [C, 8], BF, isOutput=False)
    wqT = nc.declare_dram_parameter("wqT", [128, 66 * C], BF, isOutput=False)
    wkT = nc.declare_dram_parameter("wkT", [128, 66 * C], BF, isOutput=False)
    bq2 = nc.declare_dram_parameter("bq2", [C], F32, isOutput=False)
    bk2 = nc.declare_dram_parameter("bk2", [C], F32, isOutput=False)
    emb = nc.declare_dram_parameter("emb", [8, HW], BF, isOutput=False)
    out = nc.declare_dram_parameter("out", [B, 8, HW], F32, isOutput=True)
    dbg = {}
    if _DEBUG:
        for name, shape in [("d_xq", [C, B, PADHW]), ("d_xk", [C, B, PADHW]),
                            ("d_vv", [8, B, HW]), ("d_qd", [C, B, MM]),
                            ("d_kd", [C, B, MM]), ("d_s0", [16, HW]),
                            ("d_r0", [16, HW]), ("d_vt0", [128, NJT * 9])]:
            dbg[name] = nc.declare_dram_parameter(name, shape, F32, isOutput=True)

    def interior(Xt, b):
        """[64, 48, 48] strided view of the padded map's valid region."""
        return bass.AP(
            tensor=Xt.tensor,
            offset=Xt.offset + b * PADHW + PAD * PADW + PAD,
            ap=[Xt.ap[0], [PADW, H], [1, H]],
        )

    def up_ap(Dt, b, nrows=8):
        """Broadcast view: D[c, b, p] -> [c, 36(p), 64(repeat)] (flat i//64)."""
        return bass.AP(
            tensor=Dt.tensor,
            offset=Dt.offset + b * MM,
            ap=[[Dt.ap[0][0], nrows], [1, MM], [0, 64]],
        )

    with tile.TileContext(nc) as tc, ExitStack() as ctx:
        # persistent pools (whole kernel)
        const = ctx.enter_context(tc.tile_pool(name="const", bufs=1))
        work = ctx.enter_context(tc.tile_pool(name="work", bufs=3))
        epool = ctx.enter_context(tc.tile_pool(name="epool", bufs=3))

        ID8 = const.tile([8, 8], BF)
        make_identity(nc, ID8)
        ONE9 = const.tile([1, 9], F32)
        nc.vector.memset(ONE9, 1.0)

        def _rep_body():
            # ---- persistent constants ----
            W1 = const.tile([C, 128], BF)
            nc.sync.dma_start(W1, w1T[:, :])
            WV = const.tile([C, 8], BF)
            nc.sync.dma_start(WV, wvT[:, :])
            BQ = const.tile([C, 1], F32)
            nc.sync.dma_start(BQ, bq2[:].rearrange("(p f) -> p f", f=1))
            BK = const.tile([C, 1], F32)
            nc.sync.dma_start(BK, bk2[:].rearrange("(p f) -> p f", f=1))
            QD = const.tile([C, B, MM], F32)
            KD = const.tile([C, B, MM], F32)
            Ss = [const.tile([16, HW], BF, name=f"S{b}") for b in range(B)]
            Rs = [const.tile([16, HW], BF, name=f"R{b}") for b in range(B)]
            VTs = [const.tile([128, NJT, 9], BF, name=f"VT{b}") for b in range(B)]

            # ---- prologue stage (scratch released before the main loops) ----
            with tc.tile_pool(name="stage", bufs=1) as stage, \
                 tc.tile_pool(name="psum_pro", bufs=1, space="PSUM") as pp:
                F = stage.tile([C, B, HW], BF)
                for b in range(B):
                    nc.sync.dma_start(F[:, b, :], f2[b, :, :])
                XQ = stage.tile([128, B, PADHW], BF)
                XK = stage.tile([128, B, PADHW], BF)
                VV = stage.tile([8, B, HW], BF)

                # zero the padding border (3 strips per (map, batch));
                # rows 64-127 hold the map shifted left by one element, so
                # their border strips sit one element earlier.
                for Xt in (XQ, XK):
                    p0 = [Xt.ap[0][0], 64]
                    p1 = bass.AP(tensor=Xt.tensor, offset=Xt.offset + 64 * Xt.ap[0][0], ap=Xt.ap).ap
                    for b in range(B):
                        base = b * PADHW
                        for shift, poff in ((0, 0), (1, 64)):
                            o = Xt.offset + poff * Xt.ap[0][0] + base
                            pap = [[Xt.ap[0][0], 64]]
                            nc.gpsimd.memset(
                                bass.AP(tensor=Xt.tensor, offset=o,
                                        ap=pap + [[1, 2 * PADW + PAD - shift]]), 0.0)
                            nc.gpsimd.memset(
                                bass.AP(tensor=Xt.tensor,
                                        offset=o + (H + PAD - 1) * PADW + PAD + H - shift,
                                        ap=pap + [[1, 2 * PADW + PAD + shift]]), 0.0)
                            nc.gpsimd.memset(
                                bass.AP(tensor=Xt.tensor,
                                        offset=o + PAD * PADW + PAD + H - shift,
                                        ap=pap + [[PADW, H - 1], [1, 2 * PAD]]), 0.0)

                # ---- qkv projection (chunks aligned to whole 48-rows) ----
                for b in range(B):
                    for (j0, nj) in QCHUNKS:
                        nrows = nj // H
                        y0 = j0 // H
                        pq = pp.tile([128, 480], F32, tag="pq", bufs=2)
                        nc.tensor.matmul(pq[:, :nj], lhsT=W1, rhs=F[:, b, j0:j0 + nj],
                                         start=True, stop=True)
                        pv = pp.tile([8, 480], F32, tag="pv", bufs=2)
                        nc.tensor.matmul(pv[:, :nj], lhsT=WV, rhs=F[:, b, j0:j0 + nj],
                                         start=True, stop=True)
                        for Xt, r0 in ((XQ, 0), (XK, 64)):
                            src = pq[r0:r0 + 64, :nj].rearrange(
                                "p (r w) -> p r w", r=nrows, w=H)
                            for shift, poff in ((0, 0), (1, 64)):
                                dst = bass.AP(
                                    tensor=Xt.tensor,
                                    offset=(Xt.offset + poff * Xt.ap[0][0]
                                            + b * PADHW
                                            + (PAD + y0) * PADW + PAD - shift),
                                    ap=[[Xt.ap[0][0], 64], [PADW, nrows], [1, H]])
                                nc.vector.tensor_copy(dst, src)
                        nc.vector.tensor_copy(VV[:, b, j0:j0 + nj], pv[:, :nj])

                # ---- strided 11x11 convs (121 accumulated taps, both batches);
                #      WQ and WK share one weight slot sequentially ----
                for (Xt, wdram, Bt, Dt) in ((XQ, wqT, BQ, QD), (XK, wkT, BK, KD)):
                    Wc = stage.tile([128, 66 * C], BF, tag="wconv", name="Wc")
                    nc.sync.dma_start(Wc, wdram[:, :])
                    acc = pp.tile([C, B, MM], F32, tag="acc")
                    slots = []
                    for ky in range(KS):
                        for pk in range(5):
                            slots.append((ky, 2 * pk, True))
                        slots.append((ky, 10, False))
                    for si, (ky, kx, paired) in enumerate(slots):
                        kp = 128 if paired else 64
                        rhs = bass.AP(
                            tensor=Xt.tensor,
                            offset=Xt.offset + ky * PADW + kx,
                            ap=[[Xt.ap[0][0], kp], [PADHW, B],
                                [STRIDE * PADW, M6], [STRIDE, M6]])
                        nc.tensor.matmul(acc, lhsT=Wc[0:kp, si * C:(si + 1) * C],
                                         rhs=rhs,
                                         start=(si == 0), stop=(si == len(slots) - 1))
                    nc.scalar.activation(Dt, acc, getattr(AF, _CONV_ACT), bias=Bt)

                # ---- vv^T (+ ones row) per unit ----
                for b in range(B):
                    VT = VTs[b]
                    nc.vector.memset(VT[:, :, 0:1], 1.0)
                    for jt in range(NJT):
                        pt = pp.tile([128, 8], BF, tag="pt", bufs=2)
                        nc.tensor.transpose(pt, VV[:, b, jt * 128:(jt + 1) * 128], ID8)
                        nc.vector.tensor_copy(VT[:, jt, 1:9], pt)

                # ---- per-unit S (rhs, i-side) and R (weights, j-side) ----
                for b in range(B):
                    S, R = Ss[b], Rs[b]
                    nc.sync.dma_start(S[0:8, :].rearrange("p (h w) -> p h w", h=H, w=H),
                                      interior(XQ, b)[0:8])
                    nc.sync.dma_start(R[0:8, :], emb[:, :])
                    # compute engines can't write partition-start 8; build the
                    # broadcast rows at partition 0 and DMA them into rows 8-15.
                    UPQ = stage.tile([8, HW], BF, tag="upq")
                    UPK = stage.tile([8, HW], BF, tag="upk")
                    nc.vector.tensor_scalar_mul(
                        UPQ.rearrange("p (a d) -> p a d", a=MM, d=64),
                        up_ap(QD, b), SCALE)
                    nc.vector.tensor_copy(
                        UPK.rearrange("p (a d) -> p a d", a=MM, d=64), up_ap(KD, b))
                    nc.sync.dma_start(S[8:16, :], UPQ)
                    nc.sync.dma_start(R[8:16, :], UPK)

                if _DEBUG:
                    nc.sync.dma_start(dbg["d_xq"][:, :, :], XQ)
                    nc.sync.dma_start(dbg["d_xk"][:, :, :], XK)
                    nc.sync.dma_start(dbg["d_vv"][:, :, :], VV)
                    nc.sync.dma_start(dbg["d_qd"][:, :, :], QD)
                    nc.sync.dma_start(dbg["d_kd"][:, :, :], KD)
                    nc.sync.dma_start(dbg["d_s0"][:, :], Ss[0])
                    nc.sync.dma_start(dbg["d_r0"][:, :], Rs[0])
                    nc.sync.dma_start(dbg["d_vt0"][:, :],
                                      VTs[0].rearrange("p a b -> p (a b)"))

            # ---- main attention loops ----
            # Flat software pipeline over (b, ichunk, jtile-pair): two
            # E-matmuls fill a 2-bank PSUM tile, ONE 1024-wide exp covers
            # both (amortizing ACT per-op overhead), and the pair's
            # O-matmuls are emitted one step later so PE never stalls on
            # the current exp.
            with tc.tile_pool(name="psum_main", bufs=1, space="PSUM") as pm:
                steps = [(b, i0, ni, jp)
                         for b in range(B)
                         for (i0, ni) in ICHUNKS
                         for jp in range(NJT // 2)]
                po_cur = [None]
                pending = [None]

                def emit_o():
                    pb_, pi0, pni, pjp, pesb = pending[0]
                    if pjp == 0:
                        po_cur[0] = pm.tile([9, 512], F32, tag="po",
                                            bufs=2, name="po")
                    po = po_cur[0]
                    VT = VTs[pb_]
                    nc.tensor.matmul(po[:, :pni], lhsT=VT[:, 2 * pjp, :],
                                     rhs=pesb[:, 0, :pni],
                                     start=(pjp == 0), stop=False)
                    nc.tensor.matmul(po[:, :pni], lhsT=VT[:, 2 * pjp + 1, :],
                                     rhs=pesb[:, 1, :pni],
                                     start=False, stop=(pjp == NJT // 2 - 1))
                    if pjp == NJT // 2 - 1:
                        rec = work.tile([1, 512], F32, tag="rec", name="rec")
                        nc.vector.reciprocal(rec[:, :pni], po[0:1, :pni])
                        pb = pm.tile([9, 512], F32, tag="po", bufs=2, name="pb")
                        nc.tensor.matmul(pb[:, :pni], lhsT=ONE9,
                                         rhs=rec[:, :pni],
                                         start=True, stop=True)
                        pbs = work.tile([9, 512], F32, tag="pbs", name="pbs")
                        nc.vector.tensor_copy(pbs[:, :pni], pb[:, :pni])
                        res = work.tile([9, 512], F32, tag="res", name="res")
                        nc.vector.tensor_mul(res[:, :pni], po[:, :pni],
                                             pbs[:, :pni])
                        nc.sync.dma_start(out[pb_, :, pi0:pi0 + pni],
                                          res[1:9, :pni])

                for step in steps:
                    b, i0, ni, jp = step
                    S, R = Ss[b], Rs[b]
                    pe2 = pm.tile([128, 2, 512], F32, tag="pe", bufs=3,
                                  name="pe2")
                    nc.tensor.matmul(pe2[:, 0, :ni],
                                     lhsT=R[:, (2 * jp) * 128:(2 * jp + 1) * 128],
                                     rhs=S[:, i0:i0 + ni],
                                     start=True, stop=True)
                    nc.tensor.matmul(pe2[:, 1, :ni],
                                     lhsT=R[:, (2 * jp + 1) * 128:(2 * jp + 2) * 128],
                                     rhs=S[:, i0:i0 + ni],
                                     start=True, stop=True)
                    esb2 = epool.tile([128, 2, 512], BF, tag="esb", bufs=6,
                                      name="esb2")
                    nc.scalar.activation(esb2[:, :, :ni], pe2[:, :, :ni], AF.Exp)
                    if pending[0] is not None:
                        emit_o()
                    pending[0] = (b, i0, ni, jp, esb2)
                emit_o()

        for _rep in range(repeat):
            _rep_body()

    if split:
        _split_waits(nc)
    return nc


def _split_waits(nc):
    """This walrus build allows at most ONE sync-wait per instruction.
    Move excess waits onto same-engine NoOps inserted just before."""
    import concourse.mybir as mybir
    ctr = 0
    for fn in nc.m.functions:
        for blk in fn.blocks:
            new = []
            for inst in blk.instructions:
                si = inst.sync_info
                waits = list(si.on_wait) if si and si.on_wait else []
                if len(waits) > 1:
                    for w in waits[:-1]:
                        ctr += 1
                        nop = mybir.InstNoOp(name=f"I-wsplit-{ctr}", ins=[], outs=[])
                        nop.engine = inst.engine
                        nop.sync_info = mybir.SyncInfo(on_wait=[w], on_update=[])
                        new.append(nop)
                    inst.sync_info = mybir.SyncInfo(
                        on_wait=[waits[-1]],
                        on_update=list(si.on_update or []))
                new.append(inst)
            blk.instructions = new


def _get_program(repeat=1):
    if repeat not in _PROGRAMS:
        _PROGRAMS[repeat] = _build_program(repeat)
    return _PROGRAMS[repeat]


def _make_in_maps(f, w_qkv, wq, bq, wk, bk, pos_h, pos_w):
    import ml_dtypes
    BF = ml_dtypes.bfloat16
    f2 = np.ascontiguousarray(f.reshape(B, C, HW)).astype(BF)
    embv = np.ascontiguousarray(
        (pos_h[:, :, None] + pos_w[:, None, :]).reshape(8, HW)).astype(BF)
    w = w_qkv[:, :, 0, 0].astype(np.float32)
    wq = wq.astype(np.float32)
    wk = wk.astype(np.float32)
    in_maps = []
    for h in range(NCORES):
        head = np.arange(h * 8, h * 8 + 8)
        rest = np.delete(np.arange(C), head)
        perm = np.concatenate([head, rest])
        w1T = np.ascontiguousarray(
            np.concatenate([w[0:C][perm].T, w[C:2 * C].T], axis=1)).astype(BF)
        wvT = np.ascontiguousarray(w[2 * C + h * 8: 2 * C + h * 8 + 8].T).astype(BF)
        def pack_taps(wp):
            # [oc', ic', ky, kx] -> [128, 66*64]: 5 (kx,kx+1) pairs + kx=10
            # single per ky row; partner tap weights sit at rows 64-127.
            w2 = np.zeros((128, 66 * C), np.float32)
            si = 0
            for ky in range(KS):
                for pk in range(5):
                    w2[0:64, si * C:(si + 1) * C] = wp[:, :, ky, 2 * pk].T
                    w2[64:128, si * C:(si + 1) * C] = wp[:, :, ky, 2 * pk + 1].T
                    si += 1
                w2[0:64, si * C:(si + 1) * C] = wp[:, :, ky, 10].T
                si += 1
            return np.ascontiguousarray(w2).astype(BF)

        wqp = wq[perm][:, perm]          # [oc', ic', ky, kx]
        wqT = pack_taps(wqp)
        wkp = wk[perm]                   # out-channels permuted, in natural
        wkT = pack_taps(wkp)
        in_maps.append({
            "f2": f2,
            "w1T": w1T,
            "wvT": wvT,
            "wqT": wqT,
            "wkT": wkT,
            "bq2": np.ascontiguousarray(bq[perm].astype(np.float32)),
            "bk2": np.ascontiguousarray(bk[perm].astype(np.float32)),
            "emb": embv,
        })
    return in_maps


def _assemble(results):
    fmap = np.empty((B, C, HW), np.float32)
    for h in range(NCORES):
        fmap[:, h * 8:(h + 1) * 8, :] = results[h]["out"]
    return fmap.reshape(B, C, H, H)


def run(trace=False, **inputs):
    """Run on hardware; returns (output, BassKernelResults)."""
    from concourse.bass_utils import run_bass_kernel_spmd
    nc = _get_program()
    in_maps = _make_in_maps(**inputs)
    res = run_bass_kernel_spmd(nc, in_maps, core_ids=list(range(NCORES)),
                               trace=trace)
    return _assemble(res.results), res


def kernel(**inputs):
    out, _ = run(trace=False, **inputs)
    return out



# revision 2
# speedup vs baseline: 1.6710x; 1.6710x over previous
"""Trainium2 Bass kernel for nn_Attention_8839042695176 (factored softmax).

Full inputs in, full output out. Core h owns attention head h (both batches).

Math per (b, h) unit, exploiting exp-separability of the positional logits
and the 64x-block structure of the upsampled conv-attention dots:
    N[i,j] = exp(ph[:,jy]q_i + dots[a(i), j>>6]) * exp(pw[:,jx]q_i)
with j = 48*jy + jx, a(i) = i>>6. Each jy-row of j crosses at most one
64-boundary, so (jy, d) pairs pack into NS=72 columns s (48 seg1 + 24 seg2):
    AD[i,s]  = exp(ph[:,jy(s)]q_i + dots[a(i), d(s)])        [i, 72]
    U[i,c,s] = sum_jx exp(pw[:,jx]q_i) * vv[c,48jy(s)+jx] * mask[jx,s]
    out[c,i] = sum_s AD[i,s]U[i,c,s] / sum_s AD[i,s]U[i,8,s]  (c=8: ones)
PE does all contractions (U via 48->324-wide matmuls per i-tile); DVE/Pool
do the [128,9,72] product; DVE tensor_reduce does the s-sum. The big
[2304,2304] exp of the baseline disappears entirely.
"""

import numpy as np

HEADS = 8
DIM_HEAD = 8
B = 2
C = 64
H = 48
HW = H * H            # 2304
KS = 11
PAD = 2
M6 = 6
MM = 36
SCALE = DIM_HEAD ** (-0.5)
NCORES = 8
NT = HW // 128        # 18 i-tiles
NS = 72               # packed (seg, jy) columns
GC1 = 0.7978845608028654          # sqrt(2/pi)
GC2 = GC1 * 0.044715

# s-index structures
_D1 = [(48 * jy) >> 6 for jy in range(48)]
_CROSS = [jy for jy in range(48) if ((48 * jy + 47) >> 6) != _D1[jy]]  # 24
JY_OF_S = list(range(48)) + _CROSS
D_OF_S = _D1 + [_D1[jy] + 1 for jy in _CROSS]

CHUNKS = [(0, 512), (512, 512), (1024, 512), (1536, 512), (2048, 256)]

_PROGRAMS = {}
# Per-tile engine assignment for the [128,9,72] product.
# MULT_MODE: 0 = DVE mult straight from PSUM (1x), 1 = ACT copy to SBUF bf16
# then DVE mult (2x mode), 2 = ACT copy then Pool mult (Pool can't read PSUM).
_MPAT = {0: 0, 1: 1, 2: 2, 3: 1, 4: 2, 5: 1, 6: 2, 7: 1, 8: 2}
MULT_MODE = [_MPAT[i % 9] for i in range(B * NT)]


def _build_program(repeat=1, split=True):
    from contextlib import ExitStack
    import concourse.bass as bass
    import concourse.mybir as mybir
    import concourse.tile as tile

    F32 = mybir.dt.float32
    BF = mybir.dt.bfloat16
    AF = mybir.ActivationFunctionType
    ALU = mybir.AluOpType

    nc = bass.Bass(trn_type="TRN2")

    f2 = nc.declare_dram_parameter("f2", [C, B * HW], BF, isOutput=False)
    w1T = nc.declare_dram_parameter("w1T", [C, 128], BF, isOutput=False)
    wvT = nc.declare_dram_parameter("wvT", [C, 8], BF, isOutput=False)
    wc2 = nc.declare_dram_parameter("wc2", [128, 121 * 16], BF, isOutput=False)
    bqk = nc.declare_dram_parameter("bqk", [16], F32, isOutput=False)
    phD = nc.declare_dram_parameter("phD", [8, NS], BF, isOutput=False)
    pw8 = nc.declare_dram_parameter("pw8", [8, H], BF, isOutput=False)
    e36 = nc.declare_dram_parameter("e36", [36, HW], BF, isOutput=False)
    sel36 = nc.declare_dram_parameter("sel36", [36, NS], BF, isOutput=False)
    mask48 = nc.declare_dram_parameter("mask48", [H, NS], BF, isOutput=False)
    outT = nc.declare_dram_parameter("outT", [B, HW, 8], F32, isOutput=True)

    def sap(t, off, dims):
        return bass.AP(tensor=t.tensor, offset=t.offset + off,
                       ap=[[t.ap[0][0], dims[0][1]] if dims[0][0] is None else dims[0]]
                       + list(dims[1:]))

    with tile.TileContext(nc) as tc, ExitStack() as ctx:
        const = ctx.enter_context(tc.tile_pool(name="const", bufs=1))
        work = ctx.enter_context(tc.tile_pool(name="work", bufs=3))

        def _rep_body():
            # ---- persistent tiles ----
            F = const.tile([C, B, HW], BF)
            W1 = const.tile([C, 128], BF)
            WC = const.tile([128, 121 * 16], BF)
            WV = const.tile([C, 8], BF)
            BQK = const.tile([16, 1], F32)
            PW = const.tile([8, H], BF)
            SEL = const.tile([36, NS], BF)
            MASK = const.tile([H, NS], BF)
            Q2 = const.tile([128, B, HW], BF)
            LB = const.tile([44, B, HW], BF)
            RHSAD = const.tile([44, B, NS], BF)
            BxT = const.tile([H, B, HW], BF)
            ADT = const.tile([128, B, NT, NS], BF)
            MP = const.tile([H, B, 9, NS], BF)
            QKD = const.tile([16, B, MM], BF)
            KD8 = const.tile([8, B, MM], BF)
            DLSb = const.tile([36, B, NS], BF)
            OUTT = const.tile([128, B, NT, 9], F32)

            nc.sync.dma_start(F, f2[:, :].rearrange("p (b j) -> p b j", b=B))
            nc.sync.dma_start(W1, w1T[:, :])
            nc.sync.dma_start(WC, wc2[:, :])
            nc.sync.dma_start(WV, wvT[:, :])
            nc.sync.dma_start(BQK, bqk[:].rearrange("(p f) -> p f", f=1))
            nc.sync.dma_start(PW, pw8[:, :])
            nc.sync.dma_start(SEL, sel36[:, :])
            nc.sync.dma_start(MASK, mask48[:, :])
            for b in range(B):
                nc.sync.dma_start(LB[8:44, b, :], e36[:, :])
                nc.sync.dma_start(RHSAD[0:8, b, :], phD[:, :])

            pM = MP.ap[0][0]
            p128 = Q2.ap[0][0]

            with tc.tile_pool(name="pro", bufs=1) as pro, \
                 tc.tile_pool(name="ppro", bufs=1, space="PSUM") as pp:
                # ---- qkv projection + staging ----
                for b in range(B):
                    for (j0, nj) in CHUNKS:
                        pq = pp.tile([128, 512], F32, tag="pq", bufs=2)
                        nc.tensor.matmul(pq[:, :nj], lhsT=W1,
                                         rhs=F[:, b, j0:j0 + nj],
                                         start=True, stop=True)
                        nc.scalar.activation(Q2[:, b, j0:j0 + nj], pq[:, :nj],
                                             AF.Copy)
                for b in range(B):
                    nc.sync.dma_start(LB[0:8, b, :], Q2[0:8, b, :])

                # ---- Mpack: vv slabs via per-jy matmuls, then mask ----
                for b in range(B):
                    MPP = pp.tile([H, 2, 512], F32, tag="mpp", bufs=1)
                    for s in range(NS):
                        jy = JY_OF_S[s]
                        bank, off = (0, s * 8) if s < 48 else (1, (s - 48) * 8)
                        dst = bass.AP(tensor=MPP.tensor,
                                      offset=MPP.offset + bank * 512 + off,
                                      ap=[[MPP.ap[0][0], H], [1, 8]])
                        nc.tensor.matmul(dst, lhsT=F[:, b, 48 * jy:48 * jy + 48],
                                         rhs=WV, start=True, stop=True)
                    mpp = MPP.ap[0][0]
                    for (bank, n, moff) in ((0, 48, 0), (1, 24, 48)):
                        out = bass.AP(tensor=MP.tensor,
                                      offset=MP.offset + b * 9 * NS + moff,
                                      ap=[[pM, H], [NS, 8], [1, n]])
                        in0 = bass.AP(tensor=MPP.tensor,
                                      offset=MPP.offset + bank * 512,
                                      ap=[[mpp, H], [1, 8], [8, n]])
                        in1 = bass.AP(tensor=MASK.tensor,
                                      offset=MASK.offset + moff,
                                      ap=[[MASK.ap[0][0], H], [0, 8], [1, n]])
                        nc.vector.tensor_mul(out, in0, in1)
                    nc.vector.tensor_copy(MP[:, b, 8, :], MASK[:, :])

                # ---- strided 11x11 convs, q+k stacked in K, b in free ----
                ACC = pp.tile([16, B, MM], F32, tag="acc", bufs=1)
                taps = [(2, 2)] + [(ky, kx) for ky in range(KS) for kx in range(KS)
                                   if (ky, kx) != (2, 2)]
                for ti, (ky, kx) in enumerate(taps):
                    oy0 = 1 if ky < 2 else 0
                    noy = 5 if (ky < 2 or ky == 10) else 6
                    ox0 = 1 if kx < 2 else 0
                    nox = 5 if (kx < 2 or kx == 10) else 6
                    rhs = bass.AP(
                        tensor=Q2.tensor,
                        offset=Q2.offset + (8 * oy0 + ky - PAD) * H
                        + (8 * ox0 + kx - PAD),
                        ap=[[p128, 128], [HW, B], [8 * H, noy], [8, nox]])
                    out = bass.AP(
                        tensor=ACC.tensor,
                        offset=ACC.offset + 6 * oy0 + ox0,
                        ap=[[ACC.ap[0][0], 16], [MM, B], [6, noy], [1, nox]])
                    tau = ky * KS + kx
                    nc.tensor.matmul(out, lhsT=WC[:, tau * 16:(tau + 1) * 16],
                                     rhs=rhs, start=(ti == 0),
                                     stop=(ti == len(taps) - 1))
                # gelu (tanh approx; the 0.5 folds into the dots scale)
                X = pro.tile([16, B * MM], F32, name="X")
                SQ = pro.tile([16, B * MM], F32, name="SQ")
                T1 = pro.tile([16, B * MM], F32, name="T1")
                T2 = pro.tile([16, B * MM], F32, name="T2")
                T3 = pro.tile([16, B * MM], F32, name="T3")
                accf = ACC.rearrange("p b m -> p (b m)")
                nc.scalar.activation(X, accf, AF.Identity, bias=BQK)
                nc.scalar.activation(SQ, accf, AF.Square, bias=BQK)
                nc.vector.tensor_scalar(T1, SQ, GC2, GC1, ALU.mult, ALU.add)
                nc.vector.tensor_mul(T2, T1, X)
                nc.scalar.activation(T3, T2, AF.Tanh)
                nc.vector.scalar_tensor_tensor(
                    QKD.rearrange("p b m -> p (b m)"), T3, 1.0, X,
                    ALU.add, ALU.mult)
                nc.sync.dma_start(KD8, QKD[8:16, :, :])

                # ---- dots^T -> DlogSel per b ----
                for b in range(B):
                    DT = pp.tile([36, NS], F32, tag="dt", bufs=1)
                    nc.tensor.matmul(DT[:, 0:36], lhsT=KD8[:, b, :],
                                     rhs=QKD[0:8, b, :], start=True, stop=True)
                    DTS = pro.tile([36, 36], BF, tag="dts", name="DTS")
                    nc.scalar.activation(DTS, DT[:, 0:36], AF.Copy,
                                         scale=SCALE * 0.25)
                    DLS = pp.tile([36, NS], F32, tag="dt", bufs=1)
                    nc.tensor.matmul(DLS, lhsT=DTS, rhs=SEL,
                                     start=True, stop=True)
                    nc.scalar.activation(DLSb[:, b, :], DLS, AF.Copy)
                    nc.sync.dma_start(RHSAD[8:44, b, :], DLSb[:, b, :])

                # ---- Bx = exp(pw^T q8) ----
                for b in range(B):
                    for (i0, ni) in CHUNKS:
                        XS = pp.tile([H, 512], F32, tag="xs", bufs=2)
                        nc.tensor.matmul(XS[:, :ni], lhsT=PW,
                                         rhs=Q2[0:8, b, i0:i0 + ni],
                                         start=True, stop=True)
                        nc.scalar.activation(BxT[:, b, i0:i0 + ni], XS[:, :ni],
                                             AF.Exp)

            # ---- main loops ----
            with tc.tile_pool(name="pmain", bufs=1, space="PSUM") as pm:
                for b in range(B):
                    # AD = exp(ph q + dots expand), 3 i-tiles per exp
                    for g in range(NT // 3):
                        ADS = pm.tile([128, 216], F32, tag="ads", bufs=2)
                        for u in range(3):
                            t = 3 * g + u
                            nc.tensor.matmul(
                                ADS[:, 72 * u:72 * u + 72],
                                lhsT=LB[:, b, 128 * t:128 * (t + 1)],
                                rhs=RHSAD[:, b, :], start=True, stop=True)
                        nc.scalar.activation(
                            ADT[:, b, 3 * g:3 * g + 3, :],
                            ADS.rearrange("p (a c) -> p a c", a=3, c=NS),
                            AF.Exp)
                    for t in range(NT):
                        UT = pm.tile([128, 2, 512], F32, tag="ut", bufs=3)
                        for bank in range(2):
                            dst = bass.AP(tensor=UT.tensor,
                                          offset=UT.offset + bank * 512,
                                          ap=[[UT.ap[0][0], 128], [1, 324]])
                            rhs = bass.AP(tensor=MP.tensor,
                                          offset=MP.offset + b * 9 * NS + 36 * bank,
                                          ap=[[pM, H], [NS, 9], [1, 36]])
                            nc.tensor.matmul(dst,
                                             lhsT=BxT[:, b, 128 * t:128 * (t + 1)],
                                             rhs=rhs, start=True, stop=True)
                        idx = b * NT + t
                        mode = MULT_MODE[idx]
                        P = work.tile([128, 9 * NS], BF, tag="prod", name="P")
                        pout = bass.AP(tensor=P.tensor, offset=P.offset,
                                       ap=[[P.ap[0][0], 128], [NS, 9], [36, 2], [1, 36]])
                        pin0 = bass.AP(tensor=ADT.tensor,
                                       offset=ADT.offset + (b * NT + t) * NS,
                                       ap=[[ADT.ap[0][0], 128], [0, 9], [36, 2], [1, 36]])
                        if mode == 0:
                            pin1 = bass.AP(tensor=UT.tensor, offset=UT.offset,
                                           ap=[[UT.ap[0][0], 128], [36, 9], [512, 2], [1, 36]])
                            nc.vector.tensor_mul(pout, pin0, pin1)
                        else:
                            US = work.tile([128, 2, 324], BF, tag="us", name="US")
                            cin = bass.AP(tensor=UT.tensor, offset=UT.offset,
                                          ap=[[UT.ap[0][0], 128], [512, 2], [1, 324]])
                            nc.scalar.activation(US, cin, AF.Copy)
                            pin1 = bass.AP(tensor=US.tensor, offset=US.offset,
                                           ap=[[US.ap[0][0], 128], [36, 9], [324, 2], [1, 36]])
                            eng = nc.gpsimd if mode == 2 else nc.vector
                            eng.tensor_mul(pout, pin0, pin1)
                        nc.vector.tensor_reduce(
                            OUTT[:, b, t, :],
                            P.rearrange("p (c s) -> p c s", c=9, s=NS),
                            mybir.AxisListType.X, ALU.add)
                    # final: divide and store (i-major; host transposes back)
                    REC = work.tile([128, NT], F32, tag="rec", name="REC")
                    den = bass.AP(tensor=OUTT.tensor,
                                  offset=OUTT.offset + b * NT * 9 + 8,
                                  ap=[[OUTT.ap[0][0], 128], [9, NT]])
                    nc.vector.reciprocal(REC, den)
                    RES = work.tile([128, NT, 8], F32, tag="res", name="RES")
                    num = bass.AP(tensor=OUTT.tensor,
                                  offset=OUTT.offset + b * NT * 9,
                                  ap=[[OUTT.ap[0][0], 128], [9, NT], [1, 8]])
                    rb = bass.AP(tensor=REC.tensor, offset=REC.offset,
                                 ap=[[REC.ap[0][0], 128], [1, NT], [0, 8]])
                    nc.vector.tensor_mul(RES, num, rb)
                    nc.sync.dma_start(
                        outT[b, :, :].rearrange("(t p) c -> p t c", t=NT, p=128),
                        RES)

        for _rep in range(repeat):
            _rep_body()

    if split:
        _split_waits(nc)
    return nc


def _split_waits(nc):
    """Walrus allows at most ONE sync-wait per instruction; move extras onto
    same-engine NoOps."""
    import concourse.mybir as mybir
    ctr = 0
    for fn in nc.m.functions:
        for blk in fn.blocks:
            new = []
            for inst in blk.instructions:
                si = inst.sync_info
                waits = list(si.on_wait) if si and si.on_wait else []
                if len(waits) > 1:
                    for w in waits[:-1]:
                        ctr += 1
                        nop = mybir.InstNoOp(name=f"I-wsplit-{ctr}", ins=[], outs=[])
                        nop.engine = inst.engine
                        nop.sync_info = mybir.SyncInfo(on_wait=[w], on_update=[])
                        new.append(nop)
                    inst.sync_info = mybir.SyncInfo(
                        on_wait=[waits[-1]],
                        on_update=list(si.on_update or []))
                new.append(inst)
            blk.instructions = new


def _get_program(repeat=1):
    if repeat not in _PROGRAMS:
        _PROGRAMS[repeat] = _build_program(repeat)
    return _PROGRAMS[repeat]


def _make_in_maps(f, w_qkv, wq, bq, wk, bk, pos_h, pos_w):
    import ml_dtypes
    BFD = ml_dtypes.bfloat16
    f = np.asarray(f, np.float32)
    w = np.asarray(w_qkv, np.float32)[:, :, 0, 0]
    wq = np.asarray(wq, np.float32)
    wk = np.asarray(wk, np.float32)
    bq = np.asarray(bq, np.float32)
    bk = np.asarray(bk, np.float32)
    pos_h = np.asarray(pos_h, np.float32)
    pos_w = np.asarray(pos_w, np.float32)

    f2 = np.ascontiguousarray(
        f.reshape(B, C, HW).transpose(1, 0, 2).reshape(C, B * HW)).astype(BFD)
    e36 = np.zeros((36, HW), np.float32)
    e36[np.arange(HW) >> 6, np.arange(HW)] = 1.0
    sel36 = np.zeros((36, NS), np.float32)
    sel36[D_OF_S, np.arange(NS)] = 1.0
    mask48 = np.zeros((H, NS), np.float32)
    for s in range(NS):
        j = 48 * JY_OF_S[s] + np.arange(48)
        mask48[:, s] = ((j >> 6) == D_OF_S[s]).astype(np.float32)
    phD = pos_h[:, JY_OF_S]

    in_maps = []
    for h in range(NCORES):
        head = np.arange(h * 8, h * 8 + 8)
        rest = np.delete(np.arange(C), head)
        perm = np.concatenate([head, rest])
        w1T = np.ascontiguousarray(
            np.concatenate([w[0:C][perm].T, w[C:2 * C].T], axis=1)).astype(BFD)
        wvT = np.ascontiguousarray(w[2 * C + 8 * h:2 * C + 8 * h + 8].T).astype(BFD)
        wqh = wq[head][:, perm]     # [8, 64, 11, 11], in-ch in q-map order
        wkh = wk[head]              # k-map in natural order
        wc2 = np.zeros((128, 121 * 16), np.float32)
        for ky in range(KS):
            for kx in range(KS):
                tau = ky * KS + kx
                wc2[0:64, tau * 16:tau * 16 + 8] = wqh[:, :, ky, kx].T
                wc2[64:128, tau * 16 + 8:tau * 16 + 16] = wkh[:, :, ky, kx].T
        in_maps.append({
            "f2": f2,
            "w1T": w1T,
            "wvT": wvT,
            "wc2": np.ascontiguousarray(wc2).astype(BFD),
            "bqk": np.ascontiguousarray(
                np.concatenate([bq[head], bk[head]])).astype(np.float32),
            "phD": np.ascontiguousarray(phD).astype(BFD),
            "pw8": np.ascontiguousarray(pos_w).astype(BFD),
            "e36": np.ascontiguousarray(e36).astype(BFD),
            "sel36": np.ascontiguousarray(sel36).astype(BFD),
            "mask48": np.ascontiguousarray(mask48).astype(BFD),
        })
    return in_maps


def _assemble(results):
    fmap = np.empty((B, C, HW), np.float32)
    for h in range(NCORES):
        fmap[:, h * 8:(h + 1) * 8, :] = results[h]["outT"].transpose(0, 2, 1)
    return fmap.reshape(B, C, H, H)


def run(trace=False, **inputs):
    from concourse.bass_utils import run_bass_kernel_spmd
    nc = _get_program()
    in_maps = _make_in_maps(**inputs)
    res = run_bass_kernel_spmd(nc, in_maps, core_ids=list(range(NCORES)),
                               trace=trace)
    return _assemble(res.results), res


def kernel(**inputs):
    out, _ = run(trace=False, **inputs)
    return out
